# revision 1
# baseline (speedup 1.0000x reference)
"""Trainium2 Bass kernel for nn_LinearSoftmaxAttention (second-order linear attention).

Math (per batch n, head h; L == S, D == M):
    Q = LN(queries)                       [L,D]
    K = LN(keys) / (3*sqrt(D)) * klen     [S,D]
    KV    = K^T V                         [D,M]
    Ksum  = sum_s K                       [D]
    KK    = K^T K                         [D,D]
    QQ    = Q^T Q                         [D,D]
    order1 = Q @ KV                       [L,M]
    norm1  = Q @ Ksum                     [L]
    u      = Q @ KK;  norm2 = rowsum(u * Q)
    tmat   = K @ (0.5*QQ); c = rowsum(tmat * K)
    order2 = c[:,None] * V
    out = (order1 + order2) / (norm1 + 0.5*norm2)[:,None]

Sharding: one (n,h) pair per NeuronCore -> 8 heads over 8 cores, no collectives.

v2 design notes (all matmul operands f16; fp32 would double every PE pass):
- LN stats via ONE grouped bn_stats per tensor + manual mean/var math
  (bn_stats 6-tuple per group: cnt/mean/M2 of even and odd elements).
- Apply uses broadcast (stride-0) APs: 2 tensor_tensor ops per tensor.
- Phase A/B is ONE matmul per row-chunk: stationary [kn|qn] [128,64],
  moving [1|v|kn|qn] [128,97] -> psumAB [64,97] holds every gram matrix
  (Ksum/KV/KK in kn rows, QQ in qn rows) accumulated over 4 chunks.
- PE transpose of [qn|kn] [128,64] -> [qnT;knT] [64,128] per chunk feeds a
  block-diagonal C/D matmul: lhsT=[qnT;knT], rhs=[sb_B | sb_A] [64,97]
  -> psumCD[:,t,:] = [0.5*tmat | norm1 | order1 | u] row-major.
- PE warm-up: 8 junk 512-col matmuls at kernel start keep the PE busy so the
  HAM clock-gate lifts (1.2 -> 2.4 GHz) before the real matmuls issue.
"""

from contextlib import ExitStack

import numpy as np

import concourse.bacc as bacc
import concourse.mybir as mybir
from concourse import tile
from concourse.bass_utils import run_bass_kernel_spmd
from concourse.masks import make_identity

# Problem constants (hardcoded per harness contract).
L = 512  # query length == key length
D = 32   # head dim == value dim
H = 8    # heads
P = 128  # SBUF partitions
T = L // P  # 4 row-chunks of 128
ALPHA = 3.0
LN_EPS = 1e-5
_INV_C2 = ALPHA * ALPHA * D  # 1/c^2 = 288 (K scale folded into sqrt)

_SUB = mybir.AluOpType.subtract
_MUL = mybir.AluOpType.mult
_ADD = mybir.AluOpType.add

# work tile free-dim layout: [1 | v | qn | kn | qn_dup | 2.0]
# [qn|kn] is the A/B stationary; [kn|qn_dup] is the transpose input;
# [kn|qn_dup|2.0] feeds ONE fused epilogue stt over [tmat|u|norm1]
_ONE, _V, _QN, _KN, _QN2, _TWO = 0, 1, 33, 65, 97, 129
WCOL = 130

KBYTES = 2 * T * D * 2 + T * 4  # k f16 + q f16 + klen f32 = 528


def _emit(ctx: ExitStack, tc: tile.TileContext, kin_d, v_d, out_d):
    nc = tc.nc
    f32 = mybir.dt.float32
    f16 = mybir.dt.float16
    u8 = mybir.dt.uint8
    X = mybir.AxisListType.X

    sbuf = ctx.enter_context(tc.tile_pool(name="sbuf", bufs=1))
    psum = ctx.enter_context(tc.tile_pool(name="psum", bufs=1, space="PSUM"))

    # ---- t=0: PE warm-up from const APs (no memsets before the DMAs so
    # first_useful_time starts at the DMA issue, not at an early memset) ----
    dummy = sbuf.tile([P, 1], f32)
    nc.scalar.activation(dummy[:], nc.const_aps.tensor(0.0, (P, 1)),
                         mybir.ActivationFunctionType.Sqrt)
    psum_w = psum.tile([8, 512], f32)
    wsrc = nc.const_aps.tensor(1.0, (P, 1), dtype=mybir.dt.bfloat16)
    for i in range(8):
        nc.tensor.matmul(psum_w[:], wsrc.to_broadcast((P, 8)),
                         wsrc.to_broadcast((P, 512)), start=True, stop=True)

    # ---- input DMAs first: k+q+klen (sync), v (scalar) ----
    kin = sbuf.tile([P, KBYTES], u8)
    vraw = sbuf.tile([P, T, D], f16)
    nc.sync.dma_start(kin[:], kin_d[:], single_packet=True)
    nc.scalar.dma_start(vraw[:].rearrange("p t d -> p (t d)"), v_d[:],
                        single_packet=True)
    # host packs [q | k | klen]; slot 0 = q, slot 1 = k
    kq = kin[:, 0:2 * T * D * 2].bitcast(f16).rearrange(
        "p (a t d) -> p a t d", a=2, d=D)
    klen = kin[:, 2 * T * D * 2:KBYTES].bitcast(f32)  # [P, T]

    # ---- constants (needed only mid-kernel; emitted after the DMAs) ----
    identity = sbuf.tile([P, P], f16)
    make_identity(nc, identity[:])
    eps_t = sbuf.tile([P, 1], f32)
    nc.gpsimd.memset(eps_t[:], LN_EPS)
    work = sbuf.tile([P, T, WCOL], f16)
    nc.gpsimd.memset(work[:, :, _ONE:_ONE + 1], 1.0)
    nc.gpsimd.memset(work[:, :, _TWO:_TWO + 1], 2.0)
    rhs_cd = sbuf.tile([64, 97], f16)
    nc.gpsimd.memset(rhs_cd[:], 0.0)

    # v -> work (gpsimd copy keeps ACT/DVE free)
    nc.gpsimd.tensor_copy(work[:, :, _V:_V + D], vraw[:])

    # ---- LayerNorm stats: grouped reduce + ACT square (k and q batched) ----
    # mean = sum/D;  var = sumsq/D - mean^2;  std' = sqrt(s*(var + eps))
    sq = sbuf.tile([P, 2, T, D], f16)
    nc.scalar.square(sq[:], kq)
    sums = sbuf.tile([P, 2, T], f32)
    nc.vector.reduce_sum(sums[:], kq, axis=X)
    ssq = sbuf.tile([P, 2, T], f32)
    nc.vector.reduce_sum(ssq[:], sq[:], axis=X)
    m2 = sbuf.tile([P, 2, T], f32)  # sums^2 = D^2 * mu^2
    nc.gpsimd.tensor_tensor(m2[:], sums[:], sums[:], _MUL)
    mu = sbuf.tile([P, 2, T], f32)
    nc.gpsimd.tensor_scalar(out=mu[:], in0=sums[:], scalar1=1.0 / D,
                            scalar2=None, op0=_MUL)
    # var FIRST on the DVE queue so the ACT sqrt launches as early as
    # possible; the centering + klen ops then fill the sqrt-wait slack
    var = sbuf.tile([P, 2, T], f32)  # D * actual variance
    nc.vector.scalar_tensor_tensor(out=var[:], in0=m2[:], scalar=-1.0 / D,
                                   in1=ssq[:], op0=_MUL, op1=_ADD)
    std = sbuf.tile([P, 2, T], f32)
    nc.scalar.activation(std[:], var[:], mybir.ActivationFunctionType.Sqrt,
                         scale=1.0 / D, bias=eps_t[:])
    # centered q|k in one op (runs while the sqrt is in flight)
    qkc = sbuf.tile([P, T, 2, D], f16)
    nc.vector.tensor_tensor(
        qkc[:], kq.transpose([0, 2, 1, 3]),
        mu[:, :, :, None].transpose([0, 2, 1, 3]).broadcast_to([P, T, 2, D]),
        _SUB)
    # klen (with 1/(alpha*sqrt(D)) folded in host-side) scales centered k
    nc.vector.tensor_tensor(qkc[:, :, 1, :], qkc[:, :, 1, :],
                            klen[:, :, None].broadcast_to([P, T, D]), _MUL)
    rs = sbuf.tile([P, 2, T], f32)
    nc.vector.reciprocal(rs[:], std[:])

    # ---- apply: [qn|kn] = qkc * rs in one op; qn_dup recomputed on DVE
    # (a 287ns DVE mult beats the 593ns gpsimd copy that gated transposes) ----
    qk_out = work[:, :, _QN:_QN + 2 * D].rearrange("p t (b d) -> p t b d", d=D)
    nc.vector.tensor_tensor(
        qk_out, qkc[:],
        rs[:, :, :, None].transpose([0, 2, 1, 3]).broadcast_to([P, T, 2, D]),
        _MUL)
    nc.vector.tensor_tensor(work[:, :, _QN2:_QN2 + D], qkc[:, :, 0, :],
                            rs[:, 0, :, None].broadcast_to([P, T, D]), _MUL)

    # ---- phase A/B: one matmul per chunk; grams accumulate in psumAB ----
    # rows 0:32 = qn^T @ [1|v|qn|kn] = [. | QV | QQ | .]
    # rows 32:64 = kn^T @ ...        = [Ksum | KV | KQ | KK]
    psum_ab = psum.tile([64, 97], f32)
    for t in range(T):
        nc.tensor.matmul(psum_ab[:], work[:, t, _QN:_QN + 2 * D],
                         work[:, t, 0:97], start=(t == 0), stop=(t == T - 1))

    # ---- transposes: [kn|qn2] [128,64] -> [knT;qnT] [64,128] per chunk;
    # one psum tile, two DVE copies (2 chunks each) so C/D t0/t1 start early
    qkT = sbuf.tile([64, L], f16)
    ptr = psum.tile([64, T, P], f16)
    qkT4 = qkT[:].rearrange("a (t p) -> a t p", p=P)
    for t in range(T):
        nc.tensor.transpose(ptr[:, t, :], work[:, t, _KN:_KN + 2 * D],
                            identity[:])
    nc.vector.tensor_copy(qkT4[:], ptr[:])

    # ---- psumAB -> rhs_cd (f16): [KV | QQ | KK | Ksum] (all plain; the
    # 1/(alpha*sqrt(D)) K-scale is folded into klen host-side, 0.5s into
    # the fused epilogue stt) ----
    # C-block rows 32:64 (vs qnT): KV->cols 0:32, KK->64:96, Ksum->96:97
    # D-block rows 0:32 (vs knT): QQ->cols 32:64
    nc.vector.tensor_copy(rhs_cd[32:64, 0:32], psum_ab[32:64, 1:33])
    nc.scalar.copy(rhs_cd[0:32, 32:64], psum_ab[0:32, 33:65])
    nc.scalar.copy(rhs_cd[32:64, 64:96], psum_ab[32:64, 65:97])
    nc.vector.tensor_copy(rhs_cd[32:64, 96:97], psum_ab[32:64, 0:1])

    # ---- phase C/D: one matmul per chunk ----
    # psumCD[:,t,:] = [order1(0:32) | tmat(32:64) | u(64:96) | norm1(96:97)]
    psum_cd = psum.tile([P, T, 97], f32)
    for t in range(T):
        nc.tensor.matmul(psum_cd[:, t, :], qkT[:, t * P:(t + 1) * P],
                         rhs_cd[:], start=True, stop=True)

    # ---- epilogue (row-major) ----
    # s = 0.5*[tmat|u|norm1] * [kn|qn2|2.0]  (one fused stt over 65 cols)
    # ch = rowsum(s[:,:32]);  nrm = rowsum(s[:,32:65]) = norm1 + 0.5*u.qn
    s = sbuf.tile([P, T, 2 * D + 1], f32)
    red = sbuf.tile([P, 2, T], f32)  # ch | nrm
    ch, nrm = red[:, 0], red[:, 1]
    nc.vector.scalar_tensor_tensor(out=s[:], in0=psum_cd[:, :, D:97],
                                   scalar=0.5, in1=work[:, :, _KN:_TWO + 1],
                                   op0=_MUL, op1=_MUL)
    nc.vector.reduce_sum(ch, s[:, :, 0:D], axis=X)
    nc.vector.reduce_sum(nrm, s[:, :, D:2 * D + 1], axis=X)
    nc.vector.reciprocal(nrm, nrm)
    # out = (order1 + ch*v) * rnorm
    m = sbuf.tile([P, T, D], f32)
    nc.gpsimd.tensor_tensor(m[:], vraw[:],
                            ch[:, :, None].broadcast_to([P, T, D]), _MUL)
    a = sbuf.tile([P, T, D], f32)
    nc.vector.tensor_tensor(a[:], m[:], psum_cd[:, :, 0:D], _ADD)
    out_sb = sbuf.tile([P, T, D], f32)
    nc.vector.tensor_tensor(out_sb[:], a[:],
                            nrm[:, :, None].broadcast_to([P, T, D]), _MUL)
    nc.sync.dma_start(out_d[:], out_sb[:].rearrange("p t d -> p (t d)"))


_CACHED = {}


def _build():
    if "nc" in _CACHED:
        return _CACHED["nc"]
    # Route every ACT func we use (Sqrt/Copy/Identity/Square) into the single
    # act-func-set containing Sqrt so Bacc inserts ONE table load.
    import concourse.hw_specs as hw_specs
    orig_tables = hw_specs.get_activation_tables

    def _tables_one_set(module_arch):
        tabs = orig_tables(module_arch)
        keep = None
        for name, funcs in tabs.items():
            names = {str(f) for f in funcs}
            if any("Sqrt" in s and "Rsqrt" not in s for s in names):
                keep = name
                break
        if keep is None:
            return tabs
        shared = {
            mybir.ActivationFunctionType.Copy,
            mybir.ActivationFunctionType.Identity,
            mybir.ActivationFunctionType.Square,
        }
        return {
            name: (funcs if name == keep else funcs - shared)
            for name, funcs in tabs.items()
        }

    bacc.get_activation_tables = _tables_one_set
    try:
        nc = bacc.Bacc("TRN2", target_bir_lowering=False, debug=False,
                       num_devices=H)
        f32 = mybir.dt.float32
        f16 = mybir.dt.float16
        u8 = mybir.dt.uint8
        kin_d = nc.dram_tensor("kin", [P, KBYTES], u8, kind="ExternalInput")
        v_d = nc.dram_tensor("vin", [P, T * D], f16, kind="ExternalInput")
        out_d = nc.dram_tensor("out", [P, T * D], f32, kind="ExternalOutput")
        with tile.TileContext(nc) as tc:
            with ExitStack() as ctx:
                _emit(ctx, tc, kin_d[:], v_d[:], out_d[:])
        nc.compile()
    finally:
        bacc.get_activation_tables = orig_tables
    _CACHED["nc"] = nc
    return nc


def _rows(x):
    # [512, 32] -> [128, 4*32] with col t*32+d = row t*128+p
    r = x.reshape(T, P, D).transpose(1, 0, 2)  # [P, T, D]
    return np.ascontiguousarray(r.reshape(P, T * D))


def _pack_maps(q, k, v, klen):
    maps = []
    for h in range(H):
        kb = _rows(k[0, :, h, :]).astype(np.float16)
        qb = _rows(q[0, :, h, :]).astype(np.float16)
        kl = np.ascontiguousarray(
            klen.reshape(T, P).T / (3.0 * np.sqrt(32.0))).astype(np.float32)
        kin = np.concatenate(
            [qb.view(np.uint8), kb.view(np.uint8), kl.view(np.uint8)], axis=1)
        maps.append({
            "kin": kin,
            "vin": _rows(v[0, :, h, :]).astype(np.float16),
        })
    return maps


def kernel(queries, keys, values, attn_mask, query_lengths, key_lengths,
           _want_profile=False, **_ignored):
    nc = _build()
    q = np.asarray(queries, dtype=np.float32)
    k = np.asarray(keys, dtype=np.float32)
    v = np.asarray(values, dtype=np.float32)
    klen = np.asarray(key_lengths, dtype=np.float32)

    in_maps = _pack_maps(q, k, v, klen)
    res = run_bass_kernel_spmd(nc, in_maps, list(range(H)),
                               trace=_want_profile)
    outs = [
        np.asarray(res.results[h]["out"]).astype(np.float32)
        .reshape(P, T, D).transpose(1, 0, 2).reshape(L, D)
        for h in range(H)
    ]
    out = np.stack(outs, axis=1)[None]
    if _want_profile:
        return out.astype(np.float32), res
    return out.astype(np.float32)

